# revision 7
# baseline (speedup 1.0000x reference)
"""Batched 20x20 SPD covariance-matrix inversion on 8 Trainium2 NeuronCores.

For each of 131072 batches: build C = exp(-1.5 * pairwise_dist(pos)) + 0.01*I
from 20 2-D points, return C^{-1}.

Strategy (per core, data-parallel over batch):
 - batch-major layout: each of 128 SBUF partitions holds M matrices' full
   20x20 (400 fp32) in the free dim; CHUNKS chunks of M per partition.
 - symmetric sweep operator (Gauss-Jordan preserving symmetry): only the
   upper triangle is updated each pivot, covered by 2-row rectangles
   (minimizes wasted elements vs per-op overhead on the DVE).
 - the covariance is built on the upper triangle only; the nugget TAU is
   not stored in the matrix but added when the pivot reciprocal is taken
   (exact: the diagonal offset rides additively through the sweep).
 - the final negate is folded into pivot 19 by reversing the subtraction;
   the lower-triangle mirror runs on the ACT engine interleaved with
   pivot 19's updates.
 - DVE does only the reciprocal, c*r scaling, and the rank-1 updates; all
   gathers (split per-rect so they complete during the previous pivot's
   updates), the nugget add, pivot-slot memset, diagonal/pivot-row/col
   writes, and the mirror run on the otherwise-idle ACT engine.  ACT ops
   are chained in emission order so the scheduler cannot interleave Sqrt
   and Exp activations (each table switch costs ~2.7us).
"""

import numpy as np

import concourse.bass as bass  # noqa: F401  (registers engine APIs)
import concourse.tile as tile
from concourse import bacc, mybir
from concourse.bass_utils import run_bass_kernel_spmd

N = 20                  # matrix dim
D = 2                   # coord dim
PHI = 1.5
TAU = 0.01
P = 128                 # SBUF partitions
N_CORES = 8
B_TOTAL = 131072
B_CORE = B_TOTAL // N_CORES   # 16384

F32 = mybir.dt.float32
AF = mybir.ActivationFunctionType
OP = mybir.AluOpType

RECT_H = 2              # rect-cover row height (2 minimizes DVE cycles @ M=32)


def emit_kernel(tc, pos_ap, out_ap, b_core, m_chunk):
    """Emit the per-core program. pos: [b_core, 40] f32, out: [b_core, 400] f32."""
    nc = tc.nc
    chunks = b_core // (P * m_chunk)
    assert b_core == P * m_chunk * chunks
    M = m_chunk
    rects = [(a, min(a + RECT_H, N)) for a in range(0, N, RECT_H)]

    def act(fn, *args, **kw):
        """Emit an ACT-engine op, chained so the scheduler keeps emission
        order (stops Sqrt/Exp table-set thrash and keeps gathers early)."""
        inst = fn(*args, **kw)
        tc.chain_iter_dep("actq", inst.ins)
        return inst

    pos_r = pos_ap.rearrange("(p c m) f -> p c (m f)", p=P, c=chunks)
    out_r = out_ap.rearrange("(p c m) f -> p c (m f)", p=P, c=chunks)

    with (
        tc.tile_pool(name="const", bufs=1) as const_pool,
        tc.tile_pool(name="pos", bufs=2) as pos_pool,
        tc.tile_pool(name="A", bufs=2) as a_pool,
        tc.tile_pool(name="dy", bufs=2) as dy_pool,
        tc.tile_pool(name="rect", bufs=2) as rect_pool,
        tc.tile_pool(name="small", bufs=2) as small_pool,
    ):
        tau_t = const_pool.tile([P, 1], F32)
        nc.vector.memset(tau_t[:, :], TAU)

        for c in range(chunks):
            pos_t = pos_pool.tile([P, M * N * D], F32)
            nc.sync.dma_start(pos_t[:, :], pos_r[:, c, :])
            posv = pos_t[:, :].rearrange("p (m i d) -> p m i d", m=M, i=N)

            A = a_pool.tile([P, M * N * N], F32)
            A4 = A[:, :].rearrange("p (m i j) -> p m i j", m=M, i=N)

            def rect_reg(r0, r1):
                return A4[:, :, r0:r1, r0:]

            # ---- covariance build (upper rects only): A = d^2 -> d -> exp ----
            for (r0, r1) in rects:
                nr, ncl = r1 - r0, N - r0
                reg = rect_reg(r0, r1)
                xi = posv[:, :, r0:r1, 0].unsqueeze(3).broadcast_to([P, M, nr, ncl])
                xj = posv[:, :, r0:, 0].unsqueeze(2).broadcast_to([P, M, nr, ncl])
                nc.vector.tensor_sub(reg, xi, xj)
                dy = dy_pool.tile([P, M * nr * ncl], F32, tag="dy")
                dyv = dy[:, :].rearrange("p (m i j) -> p m i j", m=M, i=nr)
                yi = posv[:, :, r0:r1, 1].unsqueeze(3).broadcast_to([P, M, nr, ncl])
                yj = posv[:, :, r0:, 1].unsqueeze(2).broadcast_to([P, M, nr, ncl])
                nc.vector.tensor_sub(dyv, yi, yj)
                act(nc.scalar.square, reg, reg)
                act(nc.scalar.square, dyv, dyv)
                nc.vector.tensor_add(reg, reg, dyv)

            for (r0, r1) in rects:
                act(nc.scalar.sqrt, rect_reg(r0, r1), rect_reg(r0, r1))

            # ---- sweep all 20 pivots ----
            for k in range(N):
                last = k == N - 1
                if k == 0:
                    # exp for rect 0 first so pivot 0's gather can start; the
                    # rest overlap pivot 0's updates (one Exp table set load).
                    r0_, r1_ = rects[0]
                    act(nc.scalar.activation,
                        rect_reg(r0_, r1_), rect_reg(r0_, r1_), AF.Exp, scale=-PHI)

                cK = small_pool.tile([P, M * N], F32, tag="c")
                crK = small_pool.tile([P, M * N], F32, tag="cr")
                rK = small_pool.tile([P, M], F32, tag="r")
                rT = small_pool.tile([P, M], F32, tag="rt")
                c3 = cK[:, :].rearrange("p (m i) -> p m i", m=M)
                cr3 = crK[:, :].rearrange("p (m i) -> p m i", m=M)

                # gather pivot column from upper storage (ACT).  The column
                # part is split along the rect cover so each piece only
                # depends on one rect of the previous pivot's update and the
                # whole gather completes while those updates are still
                # running.
                for (r0, r1) in rects:
                    lo, hi = r0, min(r1, k)
                    if lo >= hi:
                        break
                    act(nc.scalar.copy, c3[:, :, lo:hi], A4[:, :, lo:hi, k])
                act(nc.scalar.copy, c3[:, :, k:], A4[:, :, k, k:])

                # r = 1/(A[k,k] + TAU)  (nugget applied at pivot time)
                act(nc.scalar.activation, rT[:, :], c3[:, :, k],
                    AF.Identity, tau_t[:, :])
                act(nc.scalar.mul, c3[:, :, k], c3[:, :, k], 0.0)  # memset 0
                nc.vector.reciprocal(rK[:, :], rT[:, :])
                # diag <- -r (the final pivot's reversed update negates it)
                act(nc.scalar.mul, A4[:, :, k, k], rK[:, :], -1.0)

                if k == 0:
                    for (r0_, r1_) in rects[1:]:
                        act(nc.scalar.activation,
                            rect_reg(r0_, r1_), rect_reg(r0_, r1_), AF.Exp,
                            scale=-PHI)

                rb = rK[:, :].unsqueeze(2).broadcast_to([P, M, N])
                nc.vector.tensor_mul(cr3, c3, rb)

                if last:
                    # pivot col <- cr before the reversed update negates it
                    act(nc.scalar.copy, A4[:, :, :k, k], cr3[:, :, :k])

                # rank-1 update of the upper triangle (rect cover); at the
                # final pivot the subtraction is reversed, which emits the
                # negated matrix (= the inverse) directly.
                for ri, (r0, r1) in enumerate(rects):
                    nr, ncl = r1 - r0, N - r0
                    tmp = rect_pool.tile([P, M * nr * ncl], F32, tag="rect")
                    tv = tmp[:, :].rearrange("p (m i j) -> p m i j", m=M, i=nr)
                    cb = c3[:, :, r0:r1].unsqueeze(3).broadcast_to([P, M, nr, ncl])
                    crb = cr3[:, :, r0:].unsqueeze(2).broadcast_to([P, M, nr, ncl])
                    reg = rect_reg(r0, r1)
                    nc.vector.tensor_mul(tv, cb, crb)
                    if last:
                        nc.vector.tensor_sub(reg, tv, reg)
                    else:
                        nc.vector.tensor_sub(reg, reg, tv)
                    if last and ri > 0:
                        # mirror rows finalized by the previous rect (ACT)
                        for i in (2 * (ri - 1), 2 * (ri - 1) + 1):
                            if i < N - 1:
                                act(nc.scalar.copy,
                                    A4[:, :, i + 1 :, i], A4[:, :, i, i + 1 :])

                if not last:
                    # pivot col/row <- cr.  The rank-1 update left them
                    # unchanged (c[k] = cr[k] = 0), so writing after the
                    # rects is equivalent and runs concurrently on ACT.
                    if k:
                        act(nc.scalar.copy, A4[:, :, :k, k], cr3[:, :, :k])
                    act(nc.scalar.copy, A4[:, :, k, k + 1 :], cr3[:, :, k + 1 :])

            act(nc.scalar.copy, A4[:, :, N - 1 :, N - 2], A4[:, :, N - 2, N - 1 :])

            nc.sync.dma_start(out_r[:, c, :], A[:, :])


_CACHE = {}


def build_nc(b_core=B_CORE, m_chunk=32, num_devices=N_CORES):
    key = (b_core, m_chunk, num_devices)
    if key in _CACHE:
        return _CACHE[key]
    nc = bacc.Bacc(
        "TRN2", target_bir_lowering=False, debug=False, num_devices=num_devices
    )
    pos_d = nc.dram_tensor("pos", [b_core, N * D], F32, kind="ExternalInput")
    out_d = nc.dram_tensor("out", [b_core, N * N], F32, kind="ExternalOutput")
    with tile.TileContext(nc) as tc:
        emit_kernel(tc, pos_d.ap(), out_d.ap(), b_core, m_chunk)
    nc.compile()
    _CACHE[key] = nc
    return nc


def run(pos_full, b_core=B_CORE, m_chunk=32, n_cores=N_CORES, **kw):
    """pos_full: [n_cores*b_core, 20, 2] f32 -> [n_cores*b_core, 20, 20] f32."""
    nc = build_nc(b_core, m_chunk, n_cores)
    flat = np.ascontiguousarray(
        np.asarray(pos_full, dtype=np.float32).reshape(-1, N * D)
    )
    in_maps = [
        {"pos": flat[i * b_core : (i + 1) * b_core]} for i in range(n_cores)
    ]
    res = run_bass_kernel_spmd(nc, in_maps, core_ids=list(range(n_cores)), **kw)
    out = np.concatenate([r["out"] for r in res.results], axis=0)
    return out.reshape(-1, N, N), res


def kernel(neighbor_positions, edge_list=None):
    out, _ = run(neighbor_positions)
    return out


# revision 8
# speedup vs baseline: 1.1018x; 1.1018x over previous
"""Batched 20x20 SPD covariance-matrix inversion on 8 Trainium2 NeuronCores.

For each of 131072 batches: build C = exp(-1.5 * pairwise_dist(pos)) + 0.01*I
from 20 2-D points, return C^{-1}.

Strategy (per core, data-parallel over batch):
 - batch-major layout: each of 128 SBUF partitions holds M matrices' full
   20x20 (400 fp32) in the free dim; CHUNKS chunks of M per partition.
 - symmetric sweep operator (Gauss-Jordan preserving symmetry): only the
   upper triangle is updated each pivot, covered by 2-row rectangles
   (minimizes wasted elements vs per-op overhead on the DVE).
 - the covariance is built on the upper triangle only; the nugget TAU is
   not stored in the matrix but added when the pivot reciprocal is taken
   (exact: the diagonal offset rides additively through the sweep).
 - the final negate is folded into pivot 19 by reversing the subtraction;
   the lower-triangle mirror runs on the ACT engine interleaved with
   pivot 19's updates.
 - DVE does only the reciprocal, c*r scaling, and the rank-1 updates; all
   gathers (split per-rect so they complete during the previous pivot's
   updates), the nugget add, pivot-slot memset, diagonal/pivot-row/col
   writes, and the mirror run on the otherwise-idle ACT engine.  ACT ops
   are chained in emission order so the scheduler cannot interleave Sqrt
   and Exp activations (each table switch costs ~2.7us).
"""

import numpy as np

import concourse.bass as bass  # noqa: F401  (registers engine APIs)
import concourse.tile as tile
from concourse import bacc, mybir
from concourse.bass_utils import run_bass_kernel_spmd

N = 20                  # matrix dim
D = 2                   # coord dim
PHI = 1.5
TAU = 0.01
P = 128                 # SBUF partitions
N_CORES = 8
B_TOTAL = 131072
B_CORE = B_TOTAL // N_CORES   # 16384

F32 = mybir.dt.float32
AF = mybir.ActivationFunctionType
OP = mybir.AluOpType

RECT_H = 2              # rect-cover row height (2 minimizes DVE cycles @ M=32)


def emit_kernel(tc, pos_ap, out_ap, b_core, m_chunk):
    """Emit the per-core program. pos: [b_core, 40] f32, out: [b_core, 400] f32."""
    nc = tc.nc
    chunks = b_core // (P * m_chunk)
    assert b_core == P * m_chunk * chunks
    M = m_chunk
    rects = [(a, min(a + RECT_H, N)) for a in range(0, N, RECT_H)]

    def act(fn, *args, **kw):
        """Emit an ACT-engine op, chained so the scheduler keeps emission
        order (stops Sqrt/Exp table-set thrash and keeps gathers early)."""
        inst = fn(*args, **kw)
        tc.chain_iter_dep("actq", inst.ins)
        return inst

    pos_r = pos_ap.rearrange("(p c m) f -> p c (m f)", p=P, c=chunks)
    out_r = out_ap.rearrange("(p c m) f -> p c (m f)", p=P, c=chunks)

    with (
        tc.tile_pool(name="const", bufs=1) as const_pool,
        tc.tile_pool(name="pos", bufs=2) as pos_pool,
        tc.tile_pool(name="A", bufs=2) as a_pool,
        tc.tile_pool(name="dy", bufs=2) as dy_pool,
        tc.tile_pool(name="rect", bufs=2) as rect_pool,
        tc.tile_pool(name="small", bufs=2) as small_pool,
    ):
        tau_t = const_pool.tile([P, 1], F32)
        nc.vector.memset(tau_t[:, :], TAU)

        for c in range(chunks):
            pos_t = pos_pool.tile([P, M * N * D], F32)
            nc.sync.dma_start(pos_t[:, :], pos_r[:, c, :])
            posv = pos_t[:, :].rearrange("p (m i d) -> p m i d", m=M, i=N)

            A = a_pool.tile([P, M * N * N], F32)
            A4 = A[:, :].rearrange("p (m i j) -> p m i j", m=M, i=N)

            def rect_reg(r0, r1):
                return A4[:, :, r0:r1, r0:]

            # ---- covariance build (upper rects only): A = d^2 -> d -> exp ----
            for (r0, r1) in rects:
                nr, ncl = r1 - r0, N - r0
                reg = rect_reg(r0, r1)
                xi = posv[:, :, r0:r1, 0].unsqueeze(3).broadcast_to([P, M, nr, ncl])
                xj = posv[:, :, r0:, 0].unsqueeze(2).broadcast_to([P, M, nr, ncl])
                nc.vector.tensor_sub(reg, xi, xj)
                dy = dy_pool.tile([P, M * nr * ncl], F32, tag="dy")
                dyv = dy[:, :].rearrange("p (m i j) -> p m i j", m=M, i=nr)
                yi = posv[:, :, r0:r1, 1].unsqueeze(3).broadcast_to([P, M, nr, ncl])
                yj = posv[:, :, r0:, 1].unsqueeze(2).broadcast_to([P, M, nr, ncl])
                nc.vector.tensor_sub(dyv, yi, yj)
                act(nc.scalar.square, reg, reg)
                act(nc.scalar.square, dyv, dyv)
                nc.vector.tensor_add(reg, reg, dyv)

            for (r0, r1) in rects:
                act(nc.scalar.sqrt, rect_reg(r0, r1), rect_reg(r0, r1))

            # ---- sweep all 20 pivots ----
            # Per-pivot tiles; pivot k+1's are allocated during pivot k so
            # its gather/prep can overlap pivot k's updates.
            piv = {}

            def alloc_piv(k):
                cK = small_pool.tile([P, M * N], F32, tag="c")
                crK = small_pool.tile([P, M * N], F32, tag="cr")
                rK = small_pool.tile([P, M], F32, tag="r")
                rT = small_pool.tile([P, M], F32, tag="rt")
                piv[k] = (
                    cK[:, :].rearrange("p (m i) -> p m i", m=M),
                    crK[:, :].rearrange("p (m i) -> p m i", m=M),
                    rK, rT,
                )

            def prep_piv(k):
                """Row-part gather + nugget add + pivot-slot zero for pivot k
                (ACT).  Emitted as soon as row k's update has been emitted so
                the DVE reciprocal never waits."""
                c3, _, _, rT = piv[k]
                act(nc.scalar.copy, c3[:, :, k:], A4[:, :, k, k:])
                act(nc.scalar.activation, rT[:, :], c3[:, :, k],
                    AF.Identity, tau_t[:, :])
                act(nc.scalar.mul, c3[:, :, k], c3[:, :, k], 0.0)  # zero slot

            alloc_piv(0)
            r0_, r1_ = rects[0]
            act(nc.scalar.activation,
                rect_reg(r0_, r1_), rect_reg(r0_, r1_), AF.Exp, scale=-PHI)
            prep_piv(0)

            for k in range(N):
                last = k == N - 1
                c3, cr3, rK, rT = piv.pop(k)

                # r = 1/(A[k,k] + TAU)  (nugget applied at pivot time)
                nc.vector.reciprocal(rK[:, :], rT[:, :])
                # diag <- -r (the final pivot's reversed update negates it)
                act(nc.scalar.mul, A4[:, :, k, k], rK[:, :], -1.0)

                if k == 0:
                    for (r0_, r1_) in rects[1:]:
                        act(nc.scalar.activation,
                            rect_reg(r0_, r1_), rect_reg(r0_, r1_), AF.Exp,
                            scale=-PHI)

                rb = rK[:, :].unsqueeze(2).broadcast_to([P, M, N])
                nc.vector.tensor_mul(cr3, c3, rb)

                if last:
                    # pivot col <- cr before the reversed update negates it
                    act(nc.scalar.copy, A4[:, :, :k, k], cr3[:, :, :k])

                # rank-1 update of the upper triangle (rect cover); at the
                # final pivot the subtraction is reversed, which emits the
                # negated matrix (= the inverse) directly.  The rect holding
                # row k runs first (then the one holding row k+1) so the
                # pivot-row write and the next pivot's gather/prep (ACT)
                # complete while the remaining rects are still running.
                first = list(dict.fromkeys([k // 2, min((k + 1) // 2, len(rects) - 1)]))
                order = first + [j for j in range(len(rects)) if j not in first]
                for ri in order:
                    r0, r1 = rects[ri]
                    nr, ncl = r1 - r0, N - r0
                    tmp = rect_pool.tile([P, M * nr * ncl], F32, tag="rect")
                    tv = tmp[:, :].rearrange("p (m i j) -> p m i j", m=M, i=nr)
                    cb = c3[:, :, r0:r1].unsqueeze(3).broadcast_to([P, M, nr, ncl])
                    crb = cr3[:, :, r0:].unsqueeze(2).broadcast_to([P, M, nr, ncl])
                    reg = rect_reg(r0, r1)
                    nc.vector.tensor_mul(tv, cb, crb)
                    if last:
                        nc.vector.tensor_sub(reg, tv, reg)
                        # mirror the rows this rect finalized (ACT)
                        for i in (2 * ri, 2 * ri + 1):
                            if i < N - 1:
                                act(nc.scalar.copy,
                                    A4[:, :, i + 1 :, i], A4[:, :, i, i + 1 :])
                    else:
                        nc.vector.tensor_sub(reg, reg, tv)
                        if ri == k // 2:
                            # pivot row <- cr.  The rank-1 update left it
                            # unchanged (c[k] = cr[k] = 0), so writing after
                            # the rect is equivalent and runs on ACT.
                            act(nc.scalar.copy,
                                A4[:, :, k, k + 1 :], cr3[:, :, k + 1 :])
                        if ri == (k + 1) // 2:
                            alloc_piv(k + 1)
                            prep_piv(k + 1)

                if not last:
                    # column-part gather for pivot k+1, split along the rect
                    # cover: each piece depends on a single rect update above
                    # and completes while the later rects are still running.
                    cn3 = piv[k + 1][0]
                    for (r0, r1) in rects:
                        lo, hi = r0, min(r1, k + 1)
                        if lo >= hi:
                            break
                        act(nc.scalar.copy, cn3[:, :, lo:hi], A4[:, :, lo:hi, k + 1])
                    if k:
                        # pivot col <- cr (ACT, after all rects that touch it)
                        act(nc.scalar.copy, A4[:, :, :k, k], cr3[:, :, :k])

            nc.sync.dma_start(out_r[:, c, :], A[:, :])


_CACHE = {}


def build_nc(b_core=B_CORE, m_chunk=32, num_devices=N_CORES):
    key = (b_core, m_chunk, num_devices)
    if key in _CACHE:
        return _CACHE[key]
    nc = bacc.Bacc(
        "TRN2", target_bir_lowering=False, debug=False, num_devices=num_devices
    )
    pos_d = nc.dram_tensor("pos", [b_core, N * D], F32, kind="ExternalInput")
    out_d = nc.dram_tensor("out", [b_core, N * N], F32, kind="ExternalOutput")
    with tile.TileContext(nc) as tc:
        emit_kernel(tc, pos_d.ap(), out_d.ap(), b_core, m_chunk)
    nc.compile()
    _CACHE[key] = nc
    return nc


def run(pos_full, b_core=B_CORE, m_chunk=32, n_cores=N_CORES, **kw):
    """pos_full: [n_cores*b_core, 20, 2] f32 -> [n_cores*b_core, 20, 20] f32."""
    nc = build_nc(b_core, m_chunk, n_cores)
    flat = np.ascontiguousarray(
        np.asarray(pos_full, dtype=np.float32).reshape(-1, N * D)
    )
    in_maps = [
        {"pos": flat[i * b_core : (i + 1) * b_core]} for i in range(n_cores)
    ]
    res = run_bass_kernel_spmd(nc, in_maps, core_ids=list(range(n_cores)), **kw)
    out = np.concatenate([r["out"] for r in res.results], axis=0)
    return out.reshape(-1, N, N), res


def kernel(neighbor_positions, edge_list=None):
    out, _ = run(neighbor_positions)
    return out
